# revision 41
# baseline (speedup 1.0000x reference)
"""CPC loss kernel for Trainium2, data-parallel over 8 NeuronCores.

Math (per row x of shape [C], target t, y = x[t], C = 128, sp(d) = ln(1+e^d)):
  ce   = ln(sum_j e^{x_j}) - y
  bdc  = (P1 - ln2)/(C-1),                P1 = sum_j sp(x_j - y)
  bec  = 0.5*(SP - 2*P1 + S - C*y + ln2)/((C-1)(C-2)),
         SP = sum_{j,k in CxC} sp(x_j - x_k),  S = sum_j x_j

Pair enumeration is CIRCULANT: ordered pairs (j, (j+delta)%C), delta=1..127.
Pairing delta with C-delta and using sp(d)+sp(-d) = 2*ln(1+e^d) - d (the d's
telescope to zero over a full cyclic shift):

  SP = C*ln2 + 2*sum_{delta=1..63} sum_j sp(d_{j,delta})
            + sum_j sp(d_{j,64}),      d_{j,delta} = x_j - x_{(j+delta)%C}

No linear correction terms.  Only the SUM over rows is needed (scalar
output), so per-row quantities accumulate linearly: group products are
buffered across all 16 row-batches and a few big Ln(accum_out) instructions
run once at the end.

Work split per 128-row batch (8192 pair columns = 64 delta-blocks of 128):
  - delta 1..NSB (=46): TensorE matmul W gives e = x_{j+d} - x_j in PSUM;
    ScalarE computes sigma(e) = 1/(1+e^{d}) [one Sigmoid pass, no "+1"];
    product trees to groups of 8 run on DVE with 75% of level 1 on the
    otherwise-idle GPSIMD engine; ln sigma = -sp(d).
  - delta NSB+1..64: "rank-1" path with NO ScalarE work per pair:
    u = e^{x_j}*e^{-x_{j+d}} from precomputed a = e^x, b2 = [e^-x, e^-x]
    via a broadcast AP times a sliding-window AP (one DVE mul), then
    w = (1+u)*e^-LAM in one fused 4x tensor_scalar; groups of 8.
    delta=64 (weight 1, not 2) gets its own sub-tree and accumulator.
  - Group-of-8 products centered by e^-LAM stay inside the ACT Ln table's
    ~+-44.4 domain (beyond it the table clamps low / corrupts high).
  - P1 via the same rank-1 trick with b_t = e^{-y-LAM} per-partition scalar.
  - CE: ln(sum_j a_j) - y.
Tables: exp+ln in one set (natural_log_exp_and_others via the chooser
patch), Sigmoid in sigmoid_and_others -> exactly 3 table loads.
Output: per-partition partial sums [P,1]; host sums 128*8 values.
"""

import functools

import numpy as np
import ml_dtypes

import concourse.bass as bass
import concourse.tile as tile
import concourse.hw_specs as hw_specs
from concourse import bacc, mybir
from concourse.ap import AP
from concourse.bass_utils import run_bass_kernel_spmd

_orig_get_activation_tables = hw_specs.get_activation_tables


@functools.cache
def _patched_activation_tables(module_arch: str):
    d = dict(_orig_get_activation_tables(module_arch))
    for name in ("exp_and_others", "natural_log", "exp_and_friends"):
        if name in d:
            d[name] = set()
    return d


hw_specs.get_activation_tables = _patched_activation_tables
bacc.get_activation_tables = _patched_activation_tables

N, C = 16384, 128
NCORES = 8
ROWS = N // NCORES            # rows per core
P = 128                       # partitions / rows per batch
NB = ROWS // P                # batches per core
MM_N = 512                    # moving free dim per matmul (1 PSUM bank)

F32 = mybir.dt.float32
BF16 = mybir.dt.bfloat16
AF = mybir.ActivationFunctionType
ALU = mybir.AluOpType

# ---- tunables (scanned via TimelineSim) ----
NSB = 46                      # sigma-path delta blocks (delta 1..NSB)
SL1F = 0.75                   # pool fraction of each sigma-chunk level-1
R1F = 0.0                     # pool fraction of rank-1 seg2 level-1
UF = 0.0                      # pool fraction of the rank-1 u-multiply
PL_POOL = ()                  # post-loop ops on pool: subset of
                              # {"xm", "bm", "xs", "se", "mask"}
MID_RED_B = None              # emit SE/XS reduces after this batch (None=post)
WORK_BUFS = 4                 # work pool depth

LAM = 4.4
ELAM = float(np.exp(-LAM))
LOG2 = float(np.log(2.0))
M2 = (C - 1) * (C - 2)

_cache: dict = {}


def _derived():
    RB = 64 - NSB                      # rank-1 delta blocks
    scols = NSB * C                    # sigma columns
    sizes = []
    left = scols
    while left > 0:
        sizes.append(min(2048, left))
        left -= 2048
    rcols = RB * C                     # rank-1 columns
    seg2 = (RB - 1) * C                # weight-2 segment
    return RB, scols, sizes, rcols, seg2


def _consts():
    RB, scols, sizes, rcols, seg2 = _derived()
    K1 = NB * seg2 * LAM
    K64 = NB * C * LAM
    KP1C = NB * C * LAM
    CONST_T = 2.0 * K1 + K64 + NB * C * LOG2 + NB * LOG2
    K_CE = 1.0
    K_P1 = 1.0 / (C - 1) - 1.0 / M2
    K_R1 = 1.0 / M2
    K_S = -1.0 / M2
    K_R64 = 0.5 / M2
    K_SX = 0.5 / M2
    K_Y = -1.0 - 0.5 * C / M2
    CONST_L = -NB * LOG2 / (C - 1) + 0.5 * CONST_T / M2 + K_P1 * KP1C
    return [K_CE, K_P1, K_R1, K_S, K_R64, K_SX, K_Y, CONST_L]


def _build_program() -> bass.Bass:
    RB, scols, sizes, rcols, seg2 = _derived()
    nc = bacc.Bacc("TRN2")

    x_d = nc.declare_dram_parameter("x", [ROWS, C], F32, isOutput=False)
    xt_d = nc.declare_dram_parameter("xt", [C, ROWS], BF16, isOutput=False)
    w_d = nc.declare_dram_parameter("w", [C, scols], BF16, isOutput=False)
    io_d = nc.declare_dram_parameter("io", [P, C], BF16, isOutput=False)
    cf_d = nc.declare_dram_parameter("cf", [P, 8], F32, isOutput=False)
    tf_d = nc.declare_dram_parameter("tf", [ROWS], F32, isOutput=False)
    out_d = nc.declare_dram_parameter("out", [P, 1], F32, isOutput=True)

    with tile.TileContext(nc) as tc:
        with (
            tc.tile_pool(name="const", bufs=1) as const_pool,
            tc.tile_pool(name="work", bufs=WORK_BUFS) as work,
            tc.tile_pool(name="acc", bufs=1) as acc_pool,
            tc.tile_pool(name="psum", bufs=2, space="PSUM") as psum_pool,
        ):
            # x on the DVE DMA queue, xt/w on the sync queue: x (feeding the
            # exps) streams in parallel with the matmul operands, so batch
            # 0's sigma pipeline isn't serialized behind it
            HB = NB // 2
            x_sb = const_pool.tile([P, NB, C], F32)
            x_r = x_d.rearrange("(b p) c -> p b c", p=P)
            nc.sync.dma_start(out=x_sb[:, :HB, :], in_=x_r[:, :HB, :])
            # second half in parallel on the gpsimd (SWDGE) queue
            nc.gpsimd.dma_start(out=x_sb[:, HB:, :], in_=x_r[:, HB:, :])
            xt_sb = const_pool.tile([C, ROWS], BF16)
            nc.sync.dma_start(out=xt_sb, in_=xt_d[:])
            w_sb = const_pool.tile([C, scols], BF16)
            off = 0
            for sz in sizes:
                nc.sync.dma_start(
                    out=w_sb[:, off : off + sz], in_=w_d[:, off : off + sz]
                )
                off += sz
            io_sb = const_pool.tile([P, C], BF16)
            nc.sync.dma_start(out=io_sb, in_=io_d[:])
            cf_sb = const_pool.tile([P, 8], F32)
            nc.sync.dma_start(out=cf_sb, in_=cf_d[:])
            t_sb = const_pool.tile([P, NB], F32)
            nc.sync.dma_start(out=t_sb, in_=tf_d.rearrange("(b p) -> p b", p=P))

            a_sb = acc_pool.tile([P, NB, C], BF16)       # e^x
            b2 = acc_pool.tile([P, NB, 2 * C], BF16)     # [e^-x, e^-x]
            bt = acc_pool.tile([P, NB], F32)             # e^{-y-LAM} per row
            Y = acc_pool.tile([P, NB], F32)              # y per row
            SE = acc_pool.tile([P, NB], F32)             # sum_j e^x per row
            XS = acc_pool.tile([P, NB], F32)             # sum_j x per row
            gs_all = acc_pool.tile([P, NB, scols // 8], BF16)
            r1_all = acc_pool.tile([P, NB, seg2 // 8], BF16)
            r64_all = acc_pool.tile([P, NB, 16], BF16)
            p1_all = acc_pool.tile([P, NB, 16], BF16)
            ACCS = acc_pool.tile([P, 8], F32)            # CE,P1,R1,S,R64,SX,Y,1
            mask_all = acc_pool.tile([P, NB, C], BF16)
            Lfin = acc_pool.tile([P, 1], F32)

            # ---- phase E: just the exps (exp table on ACT), halved to chase
            # the split x DMA, so batch 0's pipeline starts early
            nc.scalar.activation(a_sb[:, :HB, :], x_sb[:, :HB, :], AF.Exp)
            nc.scalar.activation(
                b2[:, :HB, 0:C], x_sb[:, :HB, :], AF.Exp, bias=0.0, scale=-1.0
            )
            nc.scalar.activation(a_sb[:, HB:, :], x_sb[:, HB:, :], AF.Exp)
            nc.scalar.activation(
                b2[:, HB:, 0:C], x_sb[:, HB:, :], AF.Exp, bias=0.0, scale=-1.0
            )
            nc.vector.tensor_copy(b2[:, :, C : 2 * C], b2[:, :, 0:C])

            def emit_se_xs():
                seng = nc.gpsimd if "se" in PL_POOL else nc.vector
                seng.tensor_reduce(
                    SE, a_sb, axis=mybir.AxisListType.X, op=ALU.add
                )
                xseng = nc.gpsimd if "xs" in PL_POOL else nc.vector
                xseng.tensor_reduce(
                    XS, x_sb, axis=mybir.AxisListType.X, op=ALU.add
                )

            # ---- phase S: per-batch pair work (sigma table on ACT)
            for b in range(NB):
                if b == MID_RED_B:
                    emit_se_xs()
                lhsT = xt_sb[:, b * P : (b + 1) * P]

                goff = 0
                for ci, sz in enumerate(sizes):
                    pt = psum_pool.tile([P, 2048], F32, tag="pt")
                    for m in range(sz // MM_N):
                        f0 = sum(sizes[:ci]) + m * MM_N
                        nc.tensor.matmul(
                            pt[:, m * MM_N : (m + 1) * MM_N],
                            lhsT,
                            w_sb[:, f0 : f0 + MM_N],
                        )
                    sg = work.tile([P, 2048], BF16, tag="sg")
                    nc.scalar.activation(sg[:, :sz], pt[:, :sz], AF.Sigmoid)
                    h, q, g = sz // 2, sz // 4, sz // 8
                    # level 1 split between pool (low part) and dve
                    hh = (int(h * SL1F) // 64) * 64
                    if hh > 0:
                        nc.gpsimd.tensor_mul(
                            sg[:, :hh], sg[:, :hh], sg[:, h : h + hh]
                        )
                    if hh < h:
                        nc.vector.tensor_mul(
                            sg[:, hh:h], sg[:, hh:h], sg[:, h + hh : sz]
                        )
                    nc.vector.tensor_mul(sg[:, :q], sg[:, :q], sg[:, q:h])
                    nc.vector.tensor_mul(
                        gs_all[:, b, goff : goff + g], sg[:, :q // 2], sg[:, q // 2 : q]
                    )
                    goff += g

                # rank-1 chunk: delta NSB+1..64
                u = work.tile([P, rcols], BF16, tag="u")
                u_ap = u[:]
                u3 = AP(u_ap.tensor, u_ap.offset, [u_ap.ap[0], [C, RB], [1, C]])
                ab = a_sb[:, b, :].unsqueeze(1).broadcast_to([P, RB, C])
                bb = b2[:, b, :]
                bwin = AP(
                    bb.tensor, bb.offset + NSB + 1, [bb.ap[0], [1, RB], [1, C]]
                )
                # u-mul split between pool (leading blocks) and dve
                ub = int(RB * UF)
                if ub > 0:
                    u3p = AP(u_ap.tensor, u_ap.offset, [u_ap.ap[0], [C, ub], [1, C]])
                    abp = a_sb[:, b, :].unsqueeze(1).broadcast_to([P, ub, C])
                    bwinp = AP(
                        bb.tensor, bb.offset + NSB + 1, [bb.ap[0], [1, ub], [1, C]]
                    )
                    nc.gpsimd.tensor_mul(u3p, abp, bwinp)
                if ub < RB:
                    u3d = AP(
                        u_ap.tensor, u_ap.offset + ub * C,
                        [u_ap.ap[0], [C, RB - ub], [1, C]],
                    )
                    abd = (
                        a_sb[:, b, :].unsqueeze(1).broadcast_to([P, RB - ub, C])
                    )
                    bwind = AP(
                        bb.tensor, bb.offset + NSB + 1 + ub,
                        [bb.ap[0], [1, RB - ub], [1, C]],
                    )
                    nc.vector.tensor_mul(u3d, abd, bwind)
                # w = (1+u)*e^-LAM, one fused 4x op
                nc.vector.tensor_scalar(u, u, ELAM, ELAM, op0=ALU.mult, op1=ALU.add)
                # delta NSB+1..63 (cols 0:seg2), groups of 8
                s2, s4, s8 = seg2 // 2, seg2 // 4, seg2 // 8
                r1h = (int(s2 * R1F) // 32) * 32
                if r1h > 0:
                    nc.gpsimd.tensor_mul(
                        u[:, :r1h], u[:, :r1h], u[:, s2 : s2 + r1h]
                    )
                if r1h < s2:
                    nc.vector.tensor_mul(
                        u[:, r1h:s2], u[:, r1h:s2], u[:, s2 + r1h : seg2]
                    )
                nc.vector.tensor_mul(u[:, :s4], u[:, :s4], u[:, s4:s2])
                nc.vector.tensor_mul(r1_all[:, b, :], u[:, :s8], u[:, s8:s4])
                # delta 64 (last 128 cols), groups of 8, weight 1
                e0 = seg2
                nc.vector.tensor_mul(
                    u[:, e0 : e0 + 64], u[:, e0 : e0 + 64], u[:, e0 + 64 : e0 + 128]
                )
                nc.vector.tensor_mul(
                    u[:, e0 : e0 + 32], u[:, e0 : e0 + 32], u[:, e0 + 32 : e0 + 64]
                )
                nc.vector.tensor_mul(
                    r64_all[:, b, :], u[:, e0 : e0 + 16], u[:, e0 + 16 : e0 + 32]
                )

            # ---- post-loop gathers + P1 — overlap the ACT-only Ln tail
            # below instead of delaying batch 0 at the start; some ops go to
            # the otherwise-idle pool engine
            meng = nc.gpsimd if "mask" in PL_POOL else nc.vector
            for b in range(NB):
                meng.tensor_scalar(
                    mask_all[:, b, :], io_sb, t_sb[:, b : b + 1], None,
                    op0=ALU.is_equal,
                )
            xm = acc_pool.tile([P, NB, C], F32)
            xeng = nc.gpsimd if "xm" in PL_POOL else nc.vector
            xeng.tensor_mul(xm, x_sb, mask_all)
            nc.vector.tensor_reduce(Y, xm, axis=mybir.AxisListType.X, op=ALU.add)
            bm = acc_pool.tile([P, NB, C], BF16)
            beng = nc.gpsimd if "bm" in PL_POOL else nc.vector
            beng.tensor_mul(bm, b2[:, :, 0:C], mask_all)
            nc.vector.tensor_reduce(bt, bm, axis=mybir.AxisListType.X, op=ALU.add)
            # bts = e^-y * e^-LAM so P1's v = a*bts + e^-LAM is centered too
            nc.vector.tensor_scalar_mul(bt, bt, ELAM)
            if MID_RED_B is None:
                emit_se_xs()
            # P1: v = (a*e^-y + 1)*e^-LAM per batch (4x fused op), then the
            # product trees batched across all 16 batches in 3 big 2x ops
            va = acc_pool.tile([P, NB, C], BF16)
            for b in range(NB):
                nc.vector.tensor_scalar(
                    va[:, b, :], a_sb[:, b, :], bt[:, b : b + 1], ELAM,
                    op0=ALU.mult, op1=ALU.add,
                )
            nc.vector.tensor_mul(
                va[:, :, 0:64], va[:, :, 0:64], va[:, :, 64:128]
            )
            nc.vector.tensor_mul(
                va[:, :, 0:32], va[:, :, 0:32], va[:, :, 32:64]
            )
            nc.vector.tensor_mul(
                p1_all[:, :, :], va[:, :, 0:16], va[:, :, 16:32]
            )

            # ---- phase L: big Lns with accumulate (ln table on ACT)
            nc.scalar.activation(
                gs_all[:, :, :], gs_all[:, :, :], AF.Ln, accum_out=ACCS[:, 3:4]
            )
            nc.scalar.activation(
                r1_all[:, :, :], r1_all[:, :, :], AF.Ln, accum_out=ACCS[:, 2:3]
            )
            nc.scalar.activation(
                r64_all[:, :, :], r64_all[:, :, :], AF.Ln, accum_out=ACCS[:, 4:5]
            )
            nc.scalar.activation(SE, SE, AF.Ln, accum_out=ACCS[:, 0:1])
            nc.scalar.activation(
                p1_all[:, :, :], p1_all[:, :, :], AF.Ln, accum_out=ACCS[:, 1:2]
            )
            nc.vector.tensor_reduce(
                ACCS[:, 5:6], XS, axis=mybir.AxisListType.X, op=ALU.add
            )
            nc.vector.tensor_reduce(
                ACCS[:, 6:7], Y, axis=mybir.AxisListType.X, op=ALU.add
            )
            nc.vector.memset(ACCS[:, 7:8], 1.0)
            nc.vector.tensor_mul(ACCS, ACCS, cf_sb)
            nc.vector.tensor_reduce(
                Lfin, ACCS, axis=mybir.AxisListType.X, op=ALU.add
            )
            nc.sync.dma_start(out=out_d[:], in_=Lfin)

    nc.compile()
    return nc


def _host_constants():
    RB, scols, sizes, rcols, seg2 = _derived()
    if _cache.get("w_nsb") != NSB:
        w = np.zeros((C, scols), np.float32)
        for d in range(1, NSB + 1):
            base = (d - 1) * C
            j = np.arange(C)
            # e = x_{(j+d)%C} - x_j  ->  sigma(e) = sigma(-d_pair)
            w[(j + d) % C, base + j] += 1.0
            w[j, base + j] -= 1.0
        _cache["w"] = w.astype(ml_dtypes.bfloat16)
        _cache["io"] = np.broadcast_to(
            np.arange(C, dtype=np.float32), (P, C)
        ).astype(ml_dtypes.bfloat16).copy()
        _cache["cf"] = np.broadcast_to(
            np.array(_consts(), np.float32), (P, 8)
        ).copy()
        _cache["w_nsb"] = NSB
    return _cache["w"], _cache["io"], _cache["cf"]


def kernel(inputs: np.ndarray, targets: np.ndarray) -> np.ndarray:
    x = np.ascontiguousarray(np.asarray(inputs, dtype=np.float32))
    t = np.asarray(targets)
    assert x.shape == (N, C) and t.shape == (N,)

    if "nc" not in _cache:
        _cache["nc"] = _build_program()
    nc = _cache["nc"]
    w, io, cf = _host_constants()

    xt = np.ascontiguousarray(x.T).astype(ml_dtypes.bfloat16)
    tf = t.astype(np.float32)

    in_maps = []
    for c in range(NCORES):
        r0, r1 = c * ROWS, (c + 1) * ROWS
        in_maps.append(
            {
                "x": np.ascontiguousarray(x[r0:r1]),
                "xt": np.ascontiguousarray(xt[:, r0:r1]),
                "w": w,
                "io": io,
                "cf": cf,
                "tf": np.ascontiguousarray(tf[r0:r1]),
            }
        )

    res = run_bass_kernel_spmd(nc, in_maps, list(range(NCORES)))
    total = 0.0
    for c in range(NCORES):
        total += np.sum(res.results[c]["out"].astype(np.float64))
    return np.float32(total / N)
